# revision 21
# baseline (speedup 1.0000x reference)
"""BiLSTM Trainium2 kernel — transposed/fused formulation (V3).

Problem: B=64, T=512, D=U=512. Two independent LSTMs (fwd on xf, bwd on xb),
outputs concatenated on the feature dim.

Sharding: direction-split x batch-split. Cores 0-3 run the forward LSTM
(16 batch rows each), cores 4-7 the backward LSTM. No collectives.

Formulation (everything transposed — batch is the PE moving/free dim):
  z^T[gate_col, b] = U^T h^T + W^T x_t^T + bias, computed as 128-col gate
  tiles (16 tiles x 4 k-chunks) of tiny fp16 matmuls with the WEIGHTS
  stationary and h^T/x^T ([128, 16]) streaming.  The W-part + bias for step
  t+1 are emitted right after the U-part of step t: no recurrence dep, so
  they fill the PE while the gate chain runs.

  Gate tiles are ordered [f|g|i|o] and z is accumulated in three separate
  PSUM tiles {f}, {g,i}, {o} so sigmoid(f) can issue after only 16 of the 64
  U-matmuls, overlapping ACT with the rest of the PE stream.  g columns are
  pre-scaled x2 host-side (tanh(x) = 2*sigmoid(2x) - 1).

  Cell update per step (default ACT_TANH=1 path; BK_IG_TANH is a custom DVE
  op registered at import time — the DVE micro-op table is per-NEFF, no
  firmware change):
    fc  = sig_f * c        (DVE stock mul, fp16 state)
    ig  = sig_i * (2*sig_g - 1)   (BK_IG_TANH, fused custom DVE)
    c'  = ig + fc          (DVE stock add, fp16)
    T   = tanh(c')         (ACT, exact)
    h   = sig_o * T (fp16) (DVE stock mul)
  (BK_ACT_TANH=0 selects an alternative all-DVE tail: clamped add +
  deg-9 odd minimax tanh polynomial in two custom DVE ops — same speed,
  slightly lower accuracy; kept as a fallback.)
  h is written into the output staging tile, which is also the next step's
  matmul rhs — no transposes anywhere.  x streams in as fp16 [128,(t,k,b)];
  h streams out as fp16 [128,(t,c,b)] every OB steps.
"""

import os
import sys

sys.path.insert(0, "/opt/trn_rl_repo")

import numpy as np
from contextlib import ExitStack

import concourse.bass as bass  # noqa: F401
import concourse.tile as tile
from concourse import bacc, mybir
from concourse.bass_utils import run_bass_kernel_spmd

B, T, D, U = 64, 512, 512, 512
G = 4 * U                      # gate width 2048
NCORE = 8
NDIR_CORES = 4                 # cores per direction
B_LOC = B // NDIR_CORES        # 16
NT = 16                        # gate tiles (G / 128)
NK = 4                         # k chunks (D / 128)
CB = NK * B_LOC                # cell free width (4 chunks x 16 batch) = 64

XB = int(os.environ.get("BK_XB", "8"))      # steps per x DMA block
OB = int(os.environ.get("BK_OB", "8"))      # steps per output DMA block
NDUM = int(os.environ.get("BK_NDUM", "0"))  # dummy filler matmuls per step
ZBUFS = int(os.environ.get("BK_ZBUFS", "2"))
POOL_FC = int(os.environ.get("BK_POOL_FC", "0"))  # fc on gpsimd
POOL_H = int(os.environ.get("BK_POOL_H", "0"))    # h-mul on gpsimd
ACT_TANH = int(os.environ.get("BK_ACT_TANH", "1"))  # cell tanh on ACT instead of poly
C16 = int(os.environ.get("BK_C16", "1"))            # fp16 cell state (ACT_TANH only)
POOL_ADD = int(os.environ.get("BK_POOL_ADD", "0"))  # c'=ig+fc on gpsimd
FPOLY = int(os.environ.get("BK_FPOLY", "0"))        # f-gate sigmoid via DVE poly

CLAMP_C = 2.6
# deg-9 odd minimax coeffs for tanh on [0, 2.6] (max abs err 2.7e-3)
TA = (0.9866325884863426, -0.278550831175462, 0.0637625184246867)
TB = (-0.008001787662182125, 0.00040027875656558184)
# deg-9 odd minimax of 0.5*tanh on [0, 3.0] (f-gate sigmoid: z/2 slot,
# sigma(z) = 0.5 + 0.5*tanh(z/2); max abs err 2.7e-3, valid |z| <= 6)
FQ = (0.48792787174597113, -0.12674844203395538, 0.02454899363829788,
      -0.0024735786129598705, 9.66817015108362e-05)

F32 = mybir.dt.float32
F16 = mybir.dt.float16
AF = mybir.ActivationFunctionType
ALU = mybir.AluOpType

_BK_OPS = None


def _register_custom_ops():
    """Register our fused DVE ops in dve_ops.OPS (idempotent)."""
    global _BK_OPS
    if _BK_OPS is not None:
        return _BK_OPS
    import concourse.dve_ops as DO
    from concourse.dve_spec import (Spec, Src0, Src1, C0, C1, C2, One,
                                    lower, minn, maxx, sq)
    from concourse.dve_uop import DveOpSpec

    have = {op.name: op for op in DO.OPS if op.name.startswith("BK_")}
    if have:
        _BK_OPS = have
        return have

    y = sq(Src0)
    y2 = y * y
    specs = {
        # ig = si * (2*sg - 1)
        "BK_IG_TANH": Spec(
            body=Src0 * (Src1 + Src1 - One),
            reference=lambda in0, in1, s0, s1, imm2: in0 * (2.0 * in1 - 1.0)),
        # c' = clamp(ig + fc, s0, s1)
        "BK_CLAMP_ADD": Spec(
            body=minn(maxx(Src0 + Src1, C0), C1),
            reference=lambda in0, in1, s0, s1, imm2: np.clip(
                in0 + in1, s0, s1)),
        # r = x * (C0 + C1 y + C2 y^2)
        "BK_TANH_A": Spec(
            body=((C0 + C1 * y) + C2 * y2) * Src0,
            reference=lambda in0, s0, s1, imm2: in0 * (
                s0 + s1 * in0 * in0 + imm2 * (in0 * in0) ** 2)),
        # T = r + (x * y^3) * (C0 + C1 y)
        "BK_TANH_B": Spec(
            body=Src1 + (Src0 * (y * y2)) * (C0 + C1 * y),
            reference=lambda in0, in1, s0, s1, imm2: in1 + in0 * (
                in0 * in0) ** 3 * (s0 + s1 * in0 * in0)),
        # out = (C0 + in0) * in1   (f-gate: fc = (0.5 + r)*c)
        "BK_HALF_MUL": Spec(
            body=(C0 + Src0) * Src1,
            reference=lambda in0, in1, s0, s1, imm2: (s0 + in0) * in1),
    }
    out = {}
    for name, spec in specs.items():
        row = DO._CUSTOM_DVE_ROW_BASE + len(DO.OPS)
        shas = {}
        for ver in ("v3", "v4"):
            tmp = DveOpSpec(name=name, opcode=row,
                            uops=lower(spec, ver=ver),
                            rd1_en=DO.has_src1(spec))
            shas[ver] = tmp.sha(ver)
        op = DO.DveOp(name, spec, subdim=False, uops_sha=shas)
        DO.OPS.append(op)
        DO.CUSTOM_DVE_SPECS[name] = spec
        DO._SUB_OPCODE_FOR_NAME[name] = row
        out[name] = op
    _BK_OPS = out
    return out


def _gate_perm_scale():
    """Tile order [f|g|i|o] (4 x 128-col tiles per gate); g scaled x2.

    Keras order along 4U: [i(0:U), f(U:2U), g(2U:3U), o(3U:4U)].
    """
    idx = []
    for g0 in (U, 2 * U, 0, 3 * U):  # f, g, i, o
        idx.append(np.arange(g0, g0 + U))
    perm = np.concatenate(idx)
    scale = np.ones(G, np.float32)
    scale[U:2 * U] = 2.0  # g (new position)
    if FPOLY:
        scale[0:U] = 0.5  # f slot holds z/2 for the poly sigmoid
    return perm, scale


# z column groups in tile space: f = tiles 0..3, g = 4..7, i = 8..11, o = 12..15
_GRP_OPTS = {
    "f_gi_o": ((0, 4), (4, 12), (12, 16)),
    "f_gio": ((0, 4), (4, 16)),
    "fgi_o": ((0, 12), (12, 16)),
    "f_g_i_o": ((0, 4), (4, 8), (8, 12), (12, 16)),
}
GRP = _GRP_OPTS[os.environ.get("BK_GRP", "f_gi_o")]


def _emit(tc, nc, xT, U16, W16, biasT, hsT, t_steps):
    b = B_LOC
    ops = _register_custom_ops()
    with ExitStack() as es:
        consts = es.enter_context(tc.tile_pool(name="consts", bufs=1))

        # DMA order = first-needed first: W-part of step 0 needs x0/w/bias;
        # the U weights are only needed once h(0) exists, so u_t streams last.
        w_t = consts.tile([128, NK, NT, 128], F16, tag="w")
        bi_t = consts.tile([NT, 128 + NT * b], F16, tag="bi")
        hz = consts.tile([128, CB], F16, tag="hz")
        nc.vector.memset(hz[:], 0.0)

        xp = es.enter_context(tc.tile_pool(name="xp", bufs=3))
        _zb = os.environ.get("BK_ZB_PER", "")
        _zbl = ([int(v) for v in _zb.split(",")] if _zb
                else [ZBUFS] * len(GRP))
        zpools = [es.enter_context(tc.tile_pool(name=f"z{gi}", bufs=_zbl[gi],
                                                space="PSUM"))
                  for gi in range(len(GRP))]
        scrp = (es.enter_context(tc.tile_pool(name="scr", bufs=1,
                                              space="PSUM"))
                if NDUM else None)
        sp = es.enter_context(tc.tile_pool(name="sig", bufs=3))
        gp = es.enter_context(tc.tile_pool(name="gates", bufs=3))
        cp = es.enter_context(tc.tile_pool(name="c", bufs=2))
        op = es.enter_context(tc.tile_pool(name="out", bufs=3))

        scr = scrp.tile([128, 512], F32, tag="scr") if NDUM else None

        n_xblk = (t_steps + XB - 1) // XB
        x_tiles = {}

        def fetch_x(blk):
            if blk >= n_xblk or blk in x_tiles:
                return
            xt = xp.tile([128, XB, NK, b], F16, tag="x", name=f"x_{blk}")
            w = XB * NK * b
            nc.sync.dma_start(out=xt, in_=xT[:, blk * w:(blk + 1) * w])
            x_tiles[blk] = xt

        nc.sync.dma_start(out=w_t, in_=W16)
        nc.sync.dma_start(out=bi_t, in_=biasT)
        bias_t = bi_t[:, 0:128]
        ind_t = bi_t[:, 128:128 + NT * b]
        fetch_x(0)
        fetch_x(1)

        # u_t streams last: step 0 has h(-1)=0 so its U-part is skipped
        # entirely, and the whole first gate chain runs while u_t loads.
        u_t = consts.tile([128, NK, NT, 128], F16, tag="u")
        nc.sync.dma_start(out=u_t, in_=U16)

        c_prev = cp.tile([128, CB],
                         F16 if (C16 and ACT_TANH) else F32,
                         tag="c", name="c_init")
        nc.vector.memset(c_prev[:], 0.0)
        h_prev = hz

        def new_ztiles(t):
            return tuple(
                zpools[gi].tile([128, (g1 - g0) * b], F32, tag=f"z{gi}",
                                name=f"z{gi}_{t}")
                for gi, (g0, g1) in enumerate(GRP))

        def emit_wpart(zt, t, final=False):
            """bias + W x_t into the three psum tiles (start of accum).
            final=True marks the last matmul of each tile with stop (used for
            step 0, whose U-part is identically zero and skipped)."""
            blk, off = divmod(t, XB)
            xt = x_tiles[blk]
            for gi, (g0, g1) in enumerate(GRP):
                z = zt[gi]
                nc.tensor.matmul(z, bias_t, ind_t[:, g0 * b:g1 * b],
                                 start=True, stop=False,
                                 skip_group_check=True)
                for c in range(NK):
                    rhs = xt[:, off, c, :]
                    for g in range(g0, g1):
                        nc.tensor.matmul(z[:, (g - g0) * b:(g - g0 + 1) * b],
                                         w_t[:, c, g, :], rhs,
                                         start=False,
                                         stop=(final and c == NK - 1),
                                         skip_group_check=True)

        z_cur = new_ztiles(0)
        emit_wpart(z_cur, 0, final=True)

        otile = op.tile([128, OB, CB], F16, tag="o", name="ot_0")

        IG = ops["BK_IG_TANH"]
        CLA = ops["BK_CLAMP_ADD"]
        PTA = ops["BK_TANH_A"]
        PTB = ops["BK_TANH_B"]
        HM = ops["BK_HALF_MUL"]

        for t in range(t_steps):
            # ---- U-part of step t (waits on h(t-1)), group order f, gi, o
            # ---- (step 0: h(-1)=0, U-part skipped; z stopped by W-part) ----
            if t > 0:
                for gi, (g0, g1) in enumerate(GRP):
                    z = z_cur[gi]
                    for c in range(NK):
                        rhs = h_prev[:, c * b:(c + 1) * b]
                        for g in range(g0, g1):
                            nc.tensor.matmul(
                                z[:, (g - g0) * b:(g - g0 + 1) * b],
                                u_t[:, c, g, :], rhs,
                                start=False, stop=(c == NK - 1),
                                skip_group_check=True)

            # ---- sigmoids (ACT), in group order ----
            sig_parts = []
            for gi, (g0, g1) in enumerate(GRP):
                if FPOLY and gi == 0:
                    continue  # f-gate handled by the DVE poly below
                sgt = sp.tile([128, (g1 - g0) * b], F16, tag=f"sig{gi}",
                              name=f"sig{gi}_{t}")
                nc.scalar.activation(sgt, z_cur[gi], AF.Sigmoid)
                sig_parts.append((g0, g1, sgt))

            def gate_slice(t0):
                # gate tiles: f=0..3, g=4..7, i=8..11, o=12..15
                for g0, g1, sgt in sig_parts:
                    if g0 <= t0 < g1:
                        return sgt[:, (t0 - g0) * b:(t0 - g0 + 4) * b]
                raise AssertionError
            sg = gate_slice(4)
            si = gate_slice(8)
            sigo = gate_slice(12)

            # ---- cell chain ----
            cdt = F16 if (C16 and ACT_TANH) else F32
            fc = gp.tile([128, CB], cdt, tag="fc", name=f"fc_{t}")
            if FPOLY:
                # fc = sigma(z_f) * c = (0.5 + 0.5*tanh(z_f/2)) * c, the
                # 0.5*tanh part as the deg-9 poly straight off the z_f psum
                fr1 = gp.tile([128, CB], F32, tag="fr1", name=f"fr1_{t}")
                nc.vector._custom_dve(PTA, out=fr1, in0=z_cur[0],
                                      s0=FQ[0], s1=FQ[1], imm2=FQ[2])
                fr2 = gp.tile([128, CB], F32, tag="fr2", name=f"fr2_{t}")
                nc.vector._custom_dve(PTB, out=fr2, in0=z_cur[0], in1=fr1,
                                      s0=FQ[3], s1=FQ[4])
                nc.vector._custom_dve(HM, out=fc, in0=fr2, in1=c_prev,
                                      s0=0.5)
            else:
                sigf = gate_slice(0)
                fc_eng = nc.gpsimd if POOL_FC else nc.vector
                fc_eng.tensor_mul(fc, sigf, c_prev)
            ig = gp.tile([128, CB], cdt, tag="ig", name=f"ig_{t}")
            nc.vector._custom_dve(IG, out=ig, in0=si, in1=sg)
            cn = cp.tile([128, CB], cdt, tag="c", name=f"c_{t}")
            tch = gp.tile([128, CB], F16, tag="tch", name=f"tch_{t}")
            if ACT_TANH:
                add_eng = nc.gpsimd if POOL_ADD else nc.vector
                add_eng.tensor_add(cn, ig, fc)
                nc.scalar.activation(tch, cn, AF.Tanh)
            else:
                nc.vector._custom_dve(CLA, out=cn, in0=ig, in1=fc,
                                      s0=-CLAMP_C, s1=CLAMP_C)
                pr = gp.tile([128, CB], F32, tag="pr", name=f"pr_{t}")
                nc.vector._custom_dve(PTA, out=pr, in0=cn,
                                      s0=TA[0], s1=TA[1], imm2=TA[2])
                nc.vector._custom_dve(PTB, out=tch, in0=cn, in1=pr,
                                      s0=TB[0], s1=TB[1])
            hsl = otile[:, t % OB, :]
            h_eng = nc.gpsimd if POOL_H else nc.vector
            h_eng.tensor_mul(hsl, sigo, tch)

            h_prev = hsl
            c_prev = cn

            # ---- output DMA every OB steps ----
            if t % OB == OB - 1 or t == t_steps - 1:
                t0 = (t // OB) * OB
                nc.sync.dma_start(
                    out=hsT[:, t0 * CB:(t + 1) * CB],
                    in_=otile[:, 0:(t - t0 + 1), :])
                if t != t_steps - 1:
                    otile = op.tile([128, OB, CB], F16, tag="o",
                                    name=f"ot_{t + 1}")

            # ---- x prefetch ----
            if t % XB == 0:
                fetch_x(t // XB + 2)

            # ---- W-part of step t+1 (PE filler, no recurrence dep) ----
            if t + 1 < t_steps:
                z_next = new_ztiles(t + 1)
                emit_wpart(z_next, t + 1)
                z_cur = z_next

            # ---- dummy PE filler to hold p-state ----
            for dmy in range(NDUM):
                nc.tensor.matmul(scr, u_t[:, 0, 0, :],
                                 u_t[:, dmy % NK, 0:4, :],
                                 start=True, stop=True,
                                 skip_group_check=True)


def build_program(t_steps=T):
    _register_custom_ops()
    nc = bacc.Bacc("TRN2", target_bir_lowering=False, debug=False,
                   num_devices=NCORE)
    xT = nc.dram_tensor("xT", [128, t_steps * NK * B_LOC], F16,
                        kind="ExternalInput").ap()
    U16 = nc.dram_tensor("U16", [128, NK, NT, 128], F16,
                         kind="ExternalInput").ap()
    W16 = nc.dram_tensor("W16", [128, NK, NT, 128], F16,
                         kind="ExternalInput").ap()
    biasT = nc.dram_tensor("biasT", [NT, 128 + NT * B_LOC], F16,
                           kind="ExternalInput").ap()
    hsT = nc.dram_tensor("hsT", [128, t_steps * CB], F16,
                         kind="ExternalOutput").ap()
    with tile.TileContext(nc) as tc:
        _emit(tc, nc, xT, U16, W16, biasT, hsT, t_steps)
    nc.compile()
    return nc


_CACHE = {}


def _get_program(t_steps=T):
    key = t_steps
    if key not in _CACHE:
        _CACHE[key] = build_program(t_steps)
    return _CACHE[key]


def make_in_maps(xf, xb, Wf, Uf, bf, Wb, Ub, bb, t_steps=T):
    perm, scale = _gate_perm_scale()
    packs = {}
    for d, (W, Urec, bias) in enumerate(((Wf, Uf, bf), (Wb, Ub, bb))):
        Wp = (W[:, perm] * scale).astype(np.float16)
        Up = (Urec[:, perm] * scale).astype(np.float16)
        bp = (bias[perm] * scale).astype(np.float16)
        # [k-chunk, 128, tile, 128] -> lhsT tiles [128(k), NK, NT, 128(m)]
        U16 = np.ascontiguousarray(
            Up.reshape(NK, 128, NT, 128).transpose(1, 0, 2, 3))
        W16 = np.ascontiguousarray(
            Wp.reshape(NK, 128, NT, 128).transpose(1, 0, 2, 3))
        biasT = bp.reshape(NT, 128)
        packs[d] = (U16, W16, biasT)
    ind = np.zeros((NT, NT, B_LOC), np.float16)
    for k in range(NT):
        ind[k, k, :] = 1.0
    ind16 = ind.reshape(NT, NT * B_LOC)
    in_maps = []
    for core in range(NCORE):
        d, j = divmod(core, NDIR_CORES)
        x = (xf if d == 0 else xb)[B_LOC * j:B_LOC * (j + 1), :t_steps]
        # xT[p, (t, k, b)] = x[b, t, k*128 + p]
        xT = np.ascontiguousarray(
            x.reshape(B_LOC, t_steps, NK, 128)
             .transpose(3, 1, 2, 0)
             .reshape(128, t_steps * NK * B_LOC)).astype(np.float16)
        U16, W16, biasT = packs[d]
        bi = np.ascontiguousarray(
            np.concatenate([biasT, ind16], axis=1))
        in_maps.append({"xT": xT, "U16": U16, "W16": W16, "biasT": bi})
    return in_maps


def kernel(xf, xb, Wf, Uf, bf, Wb, Ub, bb):
    xf = np.asarray(xf, np.float32)
    xb = np.asarray(xb, np.float32)
    Wf = np.asarray(Wf, np.float32)
    Uf = np.asarray(Uf, np.float32)
    bf = np.asarray(bf, np.float32)
    Wb = np.asarray(Wb, np.float32)
    Ub = np.asarray(Ub, np.float32)
    bb = np.asarray(bb, np.float32)

    nc = _get_program()
    in_maps = make_in_maps(xf, xb, Wf, Uf, bf, Wb, Ub, bb)
    res = run_bass_kernel_spmd(nc, in_maps, list(range(NCORE)))

    out = np.empty((B, T, 2 * U), np.float32)
    for core in range(NCORE):
        d, j = divmod(core, NDIR_CORES)
        hsv = np.asarray(res.results[core]["hsT"])  # [128, T*CB] f16
        # hsT[p, (t, c, b)] -> out[b, t, d*512 + c*128 + p]
        hs = hsv.reshape(128, T, NK, B_LOC).transpose(3, 1, 2, 0)
        out[B_LOC * j:B_LOC * (j + 1), :, U * d:U * (d + 1)] = \
            hs.reshape(B_LOC, T, U).astype(np.float32)
    return out
